# revision 30
# baseline (speedup 1.0000x reference)
"""Causal self-attention (B=2, T=2048, C=2048, 16 heads) on 8 Trainium2 cores.

Sharding: tensor-parallel over heads — 2 heads per core. Each core computes
q/k/v projections for its head group, causal attention, and a partial output
projection (row-parallel Wo); the host sums the 8 partial outputs.

Device layout notes (per core):
  - All matmuls run in bf16 (full PE rate at any moving-dim width, unlike
    f32r's >=256 requirement) with f32 PSUM accumulation. Host pre-converts
    x and the weights to bf16, halving DMA traffic as well.
  - Projections produce qT/kT in [head_dim, token] layout and v in
    [token, head_dim] layout so that attention needs no on-device transposes:
      S^T tile  = kT_tile.T @ qT_window        (matmul lhsT=kT, rhs=qT)
      P^T       = exp(S^T)  (causal-masked via affine_select on the single
                  128-col diagonal band; no row-max needed: |S| < ~5)
      outT     += v_tile.T @ P^T               (matmul lhsT=v,  rhs=P^T)
      rowsum   += ones.T @ P^T                 (matmul lhsT=ones[128,1])
    softmax normalization is folded in afterwards: outT *= bcast(1/rowsum).
  - ST/PV matmuls and exp/adds are trimmed to the causal-valid column range
    [kt_rel*128, WIN) — bf16 has no minimum moving-dim, so no clamping.
  - The output projection is software-pipelined INTO the attention stream:
    after a window's softmax normalization is emitted, its 16 y-tile thunks
    go on a fill queue that is drained one thunk per attention kt-step.
    These matmuls keep the PE busy (and at max p-state) while the ACT
    engine works through the exp chain.
  - 1/sqrt(head_dim) is folded into Wq on the host.
  - DMA triggers cost ~0.65us each on their issuing engine's queue, so
    they are spread across the sync AND gpsimd queues (weights/wo and every
    other y tile go via gpsimd). Startup transfers are split by partition
    range (descriptor count = partition rows) so the first matmul starts
    ~5us after the fixed ~7us program preamble.
"""

import math
import sys
from contextlib import ExitStack

import numpy as np

sys.path.insert(0, "/opt/trn_rl_repo")

import concourse.bass as bass  # noqa: E402
import concourse.tile as tile  # noqa: E402
from concourse import bacc, mybir  # noqa: E402

F32 = mybir.dt.float32
F32R = mybir.dt.float32r
BF16 = mybir.dt.bfloat16

# Full problem constants
B_FULL, T_FULL, C_FULL = 2, 2048, 2048
N_HEADS, HEAD_DIM = 16, 128
N_CORES = 8
H_LOC = N_HEADS // N_CORES  # 2 heads per core
C_LOC = H_LOC * HEAD_DIM  # 256 output dims per core

WIN = 512  # token window for projections / attention q-window


def build_program(Bb=B_FULL, Tt=T_FULL, Cc=C_FULL):
    """Build the single-core program (SPMD across the 8 cores).

    Per-core DRAM interface:
      xT : [Cc, Bb*Tt]  bf16 (x transposed, replicated)
      wq : [Cc, C_LOC]  bf16 (Wq rows for this core's heads, transposed,
                              pre-scaled by 1/sqrt(HEAD_DIM))
      wk : [Cc, C_LOC]  bf16
      wv : [Cc, C_LOC]  bf16
      wo : [C_LOC, Cc]  bf16 (Wo columns for this core's heads, transposed)
      y  : [Bb*Tt, Cc]  bf16 out (partial sum; host reduces over cores)
    """
    BT = Bb * Tt
    n_kc = Cc // 128  # contraction chunks for projections
    n_win = BT // WIN  # projection token windows
    n_qw = Tt // WIN  # attention q-windows per batch element
    n_bt = BT // 128  # 128-token tiles
    sub = WIN // 128  # 128-token subtiles per window (4)
    n_nw = Cc // WIN  # output-projection column windows

    nc = bacc.Bacc("TRN2", target_bir_lowering=False, debug=False,
                   num_devices=N_CORES)

    xT_ap = nc.dram_tensor("xT", [Cc, BT], BF16, kind="ExternalInput").ap()
    wq_ap = nc.dram_tensor("wq", [Cc, C_LOC], BF16, kind="ExternalInput").ap()
    wk_ap = nc.dram_tensor("wk", [Cc, C_LOC], BF16, kind="ExternalInput").ap()
    wv_ap = nc.dram_tensor("wv", [Cc, C_LOC], BF16, kind="ExternalInput").ap()
    wo_ap = nc.dram_tensor("wo", [C_LOC, Cc], BF16, kind="ExternalInput").ap()
    y_ap = nc.dram_tensor("y", [BT, Cc], BF16, kind="ExternalOutput").ap()

    with tile.TileContext(nc) as tc, ExitStack() as ctx:
        const = ctx.enter_context(tc.tile_pool(name="const", bufs=1))
        wop = ctx.enter_context(tc.tile_pool(name="wop", bufs=1))
        qkv = ctx.enter_context(tc.tile_pool(name="qkv", bufs=1))
        otp = ctx.enter_context(tc.tile_pool(name="otp", bufs=1))

        ones_f32 = const.tile([128, 1], F32, tag="ones_f32")
        nc.any.memset(ones_f32[:], 1.0)
        ones_col = const.tile([128, 1], BF16, tag="ones_col")
        nc.vector.tensor_copy(ones_col[:], ones_f32[:])
        ones_row = const.tile([1, 128], BF16, tag="ones_row")
        nc.any.memset(ones_row[:], 1.0)
        # causal-mask constants for the PE-side bias matmul (see
        # attention below); allocated here, filled in after the startup
        # DMAs so the gpsimd queue issues those triggers first
        ident = const.tile([128, 128], BF16, tag="ident")
        tri_bias = const.tile([128, 128], BF16, tag="tri_bias")

        def fill_mask_consts():
            # identity lhsT x tri_bias rhs accumulates -30 into S where
            # col < row, so exp gives ~1e-13 there; keeps the causal mask
            # off the DVE/gpsimd queues entirely
            nc.any.memset(ident[:], 1.0)
            nc.gpsimd.affine_select(
                out=ident[:], in_=ident[:],
                compare_op=mybir.AluOpType.is_equal, fill=0.0, base=0,
                pattern=[[1, 128]], channel_multiplier=-1)
            nc.any.memset(tri_bias[:], 0.0)
            nc.gpsimd.affine_select(
                out=tri_bias[:], in_=tri_bias[:],
                compare_op=mybir.AluOpType.is_ge, fill=-30.0, base=0,
                pattern=[[1, 128]], channel_multiplier=-1)

        # preload the ACT exp table while the startup DMAs are in flight
        warm = const.tile([128, 1], F32, tag="warm")
        nc.scalar.activation(warm[:], ones_f32[:],
                             mybir.ActivationFunctionType.Exp)

        # Persistent SBUF tensors
        wo_s = wop.tile([128, H_LOC, Cc], BF16, tag="wo")
        qT_s = qkv.tile([128, H_LOC, BT], BF16, tag="qT")
        kT_s = qkv.tile([128, H_LOC, BT], BF16, tag="kT")
        v_s = qkv.tile([128, n_bt, C_LOC], BF16, tag="v")
        # attention output, outT layout [d, h, token]
        ot_s = otp.tile([128, H_LOC, BT], BF16, tag="ot_s")

        # ---- Stage 1: q/k/v projections --------------------------------
        with nc.named_scope("qkv_proj"), ExitStack() as s1:
            wqkv = s1.enter_context(tc.tile_pool(name="wqkv", bufs=1))
            xpool = s1.enter_context(tc.tile_pool(name="xpool", bufs=8))
            ps_qk = s1.enter_context(
                tc.tile_pool(name="ps_qk", bufs=1, space="PSUM"))
            ps_v = s1.enter_context(
                tc.tile_pool(name="ps_v", bufs=1, space="PSUM"))

            wq_s = wqkv.tile([128, n_kc, C_LOC], BF16, tag="wq")
            wk_s = wqkv.tile([128, n_kc, C_LOC], BF16, tag="wk")
            wv_s = wqkv.tile([128, n_kc, C_LOC], BF16, tag="wv")

            def dma_weights(kc):
                # weight triggers go on the (otherwise idle) gpsimd queue:
                # each dma_start costs ~0.6us on its issuing engine, and the
                # sync queue must keep up with the x-strip triggers
                ksl = slice(kc * 128, (kc + 1) * 128)
                for ws, wa in ((wq_s, wq_ap), (wk_s, wk_ap), (wv_s, wv_ap)):
                    nc.gpsimd.dma_start(ws[:, kc, :], wa[ksl, :])

            for w in range(n_win):
                toks = slice(w * WIN, (w + 1) * WIN)
                q_ps = [ps_qk.tile([128, WIN], F32, tag=f"q{h}", name=f"q_ps{h}")
                        for h in range(H_LOC)]
                k_ps = [ps_qk.tile([128, WIN], F32, tag=f"k{h}", name=f"k_ps{h}")
                        for h in range(H_LOC)]
                v_ps = [ps_v.tile([128, C_LOC], F32, tag=f"v{j}", name=f"v_ps{j}")
                        for j in range(sub)]
                for kc in range(n_kc):
                    strip = xpool.tile([128, WIN], BF16, tag="strip")
                    if w == 0 and kc == 0:
                        # critical startup sequence, split by PARTITION
                        # ranges (DMA cost is dominated by descriptor count
                        # = partition rows) across the sync AND gpsimd
                        # trigger queues so everything lands in parallel
                        nc.sync.dma_start(wq_s[0:64, 0, :], wq_ap[0:64, :])
                        nc.sync.dma_start(wq_s[64:128, 0, :],
                                          wq_ap[64:128, :])
                        for jp in range(4):
                            ps = slice(jp * 32, (jp + 1) * 32)
                            nc.sync.dma_start(strip[ps, :], xT_ap[ps, 0:WIN])
                        nc.gpsimd.dma_start(wk_s[:, 0, :], wk_ap[0:128, :])
                        nc.gpsimd.dma_start(wv_s[:, 0, :], wv_ap[0:128, :])
                    else:
                        if w == 0:
                            # weight chunks arrive just-in-time, interleaved
                            # with the first window's strips
                            dma_weights(kc)
                        nc.sync.dma_start(strip[:],
                                          xT_ap[kc * 128:(kc + 1) * 128, toks])
                    st = (kc == 0)
                    sp = (kc == n_kc - 1)
                    for h in range(H_LOC):
                        hs = slice(h * 128, (h + 1) * 128)
                        nc.tensor.matmul(q_ps[h][:], wq_s[:, kc, hs], strip[:],
                                         start=st, stop=sp)
                        if sp:
                            # evict each PSUM tile right after its closing
                            # matmul, split across ACT and DVE, so the next
                            # window's matmuls (which reuse the banks) start
                            # sooner
                            nc.scalar.copy(qT_s[:, h, toks], q_ps[h][:])
                        nc.tensor.matmul(k_ps[h][:], wk_s[:, kc, hs], strip[:],
                                         start=st, stop=sp)
                        if sp:
                            nc.vector.tensor_copy(kT_s[:, h, toks], k_ps[h][:])
                    for j in range(sub):
                        nc.tensor.matmul(v_ps[j][:],
                                         strip[:, j * 128:(j + 1) * 128],
                                         wv_s[:, kc, :], start=st, stop=sp)
                        if sp:
                            if j % 2 == 0:
                                nc.scalar.copy(v_s[:, w * sub + j, :],
                                               v_ps[j][:])
                            else:
                                nc.vector.tensor_copy(v_s[:, w * sub + j, :],
                                                      v_ps[j][:])
                if w == 0:
                    fill_mask_consts()
                if w == 1:
                    # wo is first needed much later; issue its DMAs now on
                    # otherwise-idle queues, split column-wise
                    for hc in range(H_LOC):
                        for cw in range(n_nw):
                            cs = slice(cw * WIN, (cw + 1) * WIN)
                            nc.gpsimd.dma_start(
                                wo_s[:, hc, cs],
                                wo_ap[hc * 128:(hc + 1) * 128, cs])

        # ---- Stages 2+3: attention with the output projection pipelined
        # into the instruction stream as PE filler.
        with nc.named_scope("attention"), ExitStack() as s2:
            ptpool = s2.enter_context(tc.tile_pool(name="ptpool", bufs=4))
            accpool = s2.enter_context(tc.tile_pool(name="accpool", bufs=2))
            spool = s2.enter_context(tc.tile_pool(name="spool", bufs=2))
            ypool = s2.enter_context(tc.tile_pool(name="ypool", bufs=12))
            ps_at = s2.enter_context(
                tc.tile_pool(name="ps_at", bufs=2, space="PSUM"))

            fill_q = []  # pending out-projection y-tile thunks
            evict_rr = [0]

            def push_outproj(b, qw):
                qoff = b * Tt + qw * WIN
                for j in range(sub):
                    bt = qoff // 128 + j
                    rows = slice(bt * 128, (bt + 1) * 128)
                    for nw in range(n_nw):
                        cols = slice(nw * WIN, (nw + 1) * WIN)

                        def _thunk(rows=rows, cols=cols, split_dma=False):
                            y_ps = ps_at.tile([128, WIN], F32, tag="sty",
                                              bufs=4, name="y_ps")
                            for hc in range(H_LOC):
                                nc.tensor.matmul(y_ps[:], ot_s[:, hc, rows],
                                                 wo_s[:, hc, cols],
                                                 start=(hc == 0),
                                                 stop=(hc == H_LOC - 1))
                            y_sb = ypool.tile([128, WIN], BF16, tag="ysb")
                            # alternate eviction engine so neither ACT nor
                            # DVE saturates and gates PSUM recycling
                            r = evict_rr[0]
                            evict_rr[0] += 1
                            if r % 2 == 0:
                                nc.scalar.copy(y_sb[:], y_ps[:])
                            else:
                                nc.vector.tensor_copy(y_sb[:], y_ps[:])
                            dma_eng = nc.sync if r % 2 == 0 else nc.gpsimd
                            if split_dma:
                                half = WIN // 2
                                c0 = cols.start
                                nc.sync.dma_start(
                                    y_ap[rows, c0:c0 + half],
                                    y_sb[:, 0:half])
                                nc.gpsimd.dma_start(
                                    y_ap[rows, c0 + half:c0 + WIN],
                                    y_sb[:, half:WIN])
                            else:
                                dma_eng.dma_start(y_ap[rows, cols], y_sb[:])

                        fill_q.append(_thunk)

            def pop_fill(n=1):
                for _ in range(n):
                    if fill_q:
                        fill_q.pop(0)()

            pending_norm = []
            for b in range(Bb):
                for qw in range(n_qw):
                    # both heads interleaved: two independent ST->exp->PV
                    # chains give the PE work while the ACT exp runs
                    qoff = b * Tt + qw * WIN
                    qsl = slice(qoff, qoff + WIN)
                    n_kt = sub * (qw + 1)
                    ot_ps = [ps_at.tile([128, WIN], F32, tag="ot", bufs=4,
                                        name=f"ot_ps{h}") for h in range(H_LOC)]
                    acc = [accpool.tile([128, WIN], BF16, tag=f"acc{h}",
                                        name=f"acc{h}") for h in range(H_LOC)]

                    def col_start(kt):
                        # causal-valid columns for diagonal tiles (bf16 runs
                        # full rate at any moving-dim, so trim exactly)
                        kt_rel = kt - qw * sub
                        return 0 if kt_rel <= 0 else kt_rel * 128

                    def st_pair(kt):
                        koff = b * Tt + kt * 128
                        vs = col_start(kt)
                        masked = (kt >= qw * sub)
                        ts = []
                        for h in range(H_LOC):
                            t = ps_at.tile([128, WIN], F32, tag="sty",
                                           bufs=4, name=f"st_ps{h}")
                            nc.tensor.matmul(
                                t[:, vs:], kT_s[:, h, koff:koff + 128],
                                qT_s[:, h, qoff + vs:qoff + WIN],
                                start=True, stop=not masked)
                            if masked:
                                # causal mask: accumulate -30 into the
                                # diagonal band where col < row
                                nc.tensor.matmul(t[:, vs:vs + 128],
                                                 ident[:], tri_bias[:],
                                                 start=False, stop=True)
                            ts.append(t)
                        return ts

                    # previous window's normalization is emitted in two
                    # stages straddling this window's first ST prefetches,
                    # so its PE->DVE->PE->DVE chain hides behind real work
                    prev = pending_norm.pop(0) if pending_norm else None
                    if prev:
                        prev[1]()  # rowsums + reciprocals
                    sts = {0: st_pair(0)}
                    if prev:
                        prev[2]()  # broadcasts + normalizing muls
                        push_outproj(*prev[0])
                        pop_fill(1)
                    if n_kt > 1:
                        sts[1] = st_pair(1)
                    is_last = (b == Bb - 1 and qw == n_qw - 1)
                    for kt in range(n_kt):
                        vs = col_start(kt)
                        st_cur = sts.pop(kt)
                        if kt + 2 < n_kt:
                            sts[kt + 2] = st_pair(kt + 2)
                        first = (kt == 0)
                        last = (kt == n_kt - 1)
                        vt = b * (Tt // 128) + kt
                        pts = []
                        for h in range(H_LOC):
                            pt = ptpool.tile([128, WIN], BF16, tag="pt",
                                             name=f"pt{h}")
                            nc.scalar.activation(
                                pt[:, vs:], st_cur[h][:, vs:],
                                mybir.ActivationFunctionType.Exp)
                            pts.append(pt)
                        for h in range(H_LOC):
                            nc.tensor.matmul(ot_ps[h][:, vs:],
                                             v_s[:, vt, h * 128:(h + 1) * 128],
                                             pts[h][:, vs:],
                                             start=first, stop=last)
                            # rowsum partials accumulate on DVE (frees the
                            # PE; gpsimd tensor ops are ~4x slower, keep off)
                            if first:
                                nc.vector.tensor_copy(acc[h][:], pts[h][:])
                            else:
                                nc.vector.tensor_add(acc[h][:, vs:],
                                                     acc[h][:, vs:],
                                                     pts[h][:, vs:])
                        # drain one (or two, if backlogged) out-proj thunks
                        # per kt step: PE filler while ACT runs exp. In the
                        # last window keep two in reserve to bridge the
                        # flush's norm chain.
                        if not (is_last and len(fill_q) <= 2):
                            pop_fill(2 if len(fill_q) > 20 else 1)

                    # per head: rowsum (PE) -> reciprocal (DVE) ->
                    # partition-broadcast via K=1 f32 matmul (PE) -> ACT
                    # stage to SBUF -> normalize (DVE). No gpsimd on the
                    # chain; transient PSUM tiles borrow sty-pool banks.
                    srecs = []

                    def _norm_a(acc2=acc, srecs=srecs):
                        for h in range(H_LOC):
                            s_ps = ps_at.tile([128, WIN], F32, tag="sty",
                                              bufs=4, name="s_ps")
                            nc.tensor.matmul(s_ps[0:1, :], ones_col[:],
                                             acc2[h][:],
                                             start=True, stop=True)
                            # approx reciprocal: ~18 correct bits (rowsums
                            # are >= exp(s_ii) > 0.1), 5x faster
                            srec = spool.tile([1, WIN], F32, tag=f"srec{h}",
                                              name="srec")
                            nc.vector.reciprocal_approx_fast(srec[:],
                                                             s_ps[0:1, :])
                            srec_b = spool.tile([1, WIN], BF16,
                                                tag=f"srecb{h}", name="srec_b")
                            nc.vector.tensor_copy(srec_b[:], srec[:])
                            srecs.append(srec_b)

                    def _norm_b(ot2=ot_ps, qsl=qsl, srecs=srecs):
                        for h in range(H_LOC):
                            bc_ps = ps_at.tile([128, WIN], F32, tag="sty",
                                               bufs=4, name="bc_ps")
                            nc.tensor.matmul(bc_ps[:], ones_row[:],
                                             srecs[h][:],
                                             start=True, stop=True)
                            # DVE reads only one PSUM operand: stage the
                            # broadcast in SBUF via ACT
                            bc_sb = spool.tile([128, WIN], F32, tag=f"bc{h}",
                                               name="bc_sb")
                            nc.scalar.copy(bc_sb[:], bc_ps[:])
                            nc.vector.tensor_mul(ot_s[:, h, qsl], ot2[h][:],
                                                 bc_sb[:])

                    pending_norm.append(((b, qw), _norm_a, _norm_b))
                    # normalizations deferred by one window so the gpsimd
                    # reduce never stalls this window's mask chain; once a
                    # window is normalized its out-proj tiles join the fill
                    # queue
            # flush deferred normalizations and remaining out-proj tiles;
            # split the trailing DMAs so the drain tail stays short
            while pending_norm:
                pwin, na, nb = pending_norm.pop(0)
                na()
                pop_fill(1)
                nb()
                pop_fill(1)
                push_outproj(*pwin)
            while fill_q:
                # split only the final two transfers (2 extra triggers);
                # splitting more serializes on the trigger engines
                fill_q.pop(0)(split_dma=(len(fill_q) <= 2))

    nc.compile()
    return nc


_PROGRAM = None


def _get_program():
    global _PROGRAM
    if _PROGRAM is None:
        _PROGRAM = build_program()
    return _PROGRAM


def make_in_maps(x, Wq, Wk, Wv, Wo):
    """Host-side sharding: build the per-core input dicts (bf16)."""
    import ml_dtypes
    bf16 = ml_dtypes.bfloat16
    x = np.asarray(x, dtype=np.float32)
    Wq = np.asarray(Wq, dtype=np.float32)
    Wk = np.asarray(Wk, dtype=np.float32)
    Wv = np.asarray(Wv, dtype=np.float32)
    Wo = np.asarray(Wo, dtype=np.float32)
    BT = x.shape[0] * x.shape[1]
    xT = np.ascontiguousarray(x.reshape(BT, -1).T.astype(bf16))
    scale = 1.0 / math.sqrt(HEAD_DIM)
    in_maps = []
    for c in range(N_CORES):
        rows = slice(c * C_LOC, (c + 1) * C_LOC)
        in_maps.append({
            "xT": xT,
            "wq": np.ascontiguousarray((Wq[rows, :].T * scale).astype(bf16)),
            "wk": np.ascontiguousarray(Wk[rows, :].T.astype(bf16)),
            "wv": np.ascontiguousarray(Wv[rows, :].T.astype(bf16)),
            "wo": np.ascontiguousarray(Wo[:, rows].T.astype(bf16)),
        })
    return in_maps


def kernel(x, Wq, Wk, Wv, Wo):
    from concourse.bass_utils import run_bass_kernel_spmd

    nc = _get_program()
    in_maps = make_in_maps(x, Wq, Wk, Wv, Wo)
    res = run_bass_kernel_spmd(nc, in_maps, list(range(N_CORES)))
    x = np.asarray(x)
    Bb, Tt, Cc = x.shape
    y = np.zeros((Bb * Tt, Cc), dtype=np.float32)
    for c in range(N_CORES):
        y += np.asarray(res.results[c]["y"], dtype=np.float32)
    return y.reshape(Bb, Tt, Cc)
